# revision 1
# baseline (speedup 1.0000x reference)
"""Cost-volume kernel for Trainium2 (Bass/Tile), SPMD over 8 NeuronCores.

out[n, c, d, h, x] = l[n, c, h, x] - r[n, c, h, x - d]  for x >= d, else 1.0
shapes: l, r = (2, 32, 128, 256) f32 -> out = (2, 32, 48, 128, 256) f32

Sharding: the 64 (n, c) pairs split 8 ways -> 8 "channels" (G) per core; no
cross-core communication. Per core: h=128 is the SBUF partition dim, one DVE
subtract per disparity d covering all 8 channels, ones filled by memset, one
~1 MiB DMA per d to the contiguous (g, d) output slabs.
"""

import numpy as np

import concourse.bacc as bacc
import concourse.mybir as mybir
import concourse.tile as tile
from concourse.bass_utils import run_bass_kernel_spmd

MAX_DISP = 48
N, C, H, W = 2, 32, 128, 256
NCORES = 8
G = (N * C) // NCORES  # 8 (n, c) channels per core

_CACHE = {}


def build_bass():
    if "nc" in _CACHE:
        return _CACHE["nc"]
    nc = bacc.Bacc("TRN2", target_bir_lowering=False, debug=False)
    l = nc.dram_tensor("l", (G, H, W), mybir.dt.float32, kind="ExternalInput")
    r = nc.dram_tensor("r", (G, H, W), mybir.dt.float32, kind="ExternalInput")
    out = nc.dram_tensor(
        "out", (G, MAX_DISP, H, W), mybir.dt.float32, kind="ExternalOutput"
    )

    with tile.TileContext(nc) as tc:
        with tc.tile_pool(name="inp", bufs=1) as inpool, tc.tile_pool(
            name="outp", bufs=6
        ) as outpool:
            l_sb = inpool.tile([H, G, W], mybir.dt.float32)
            r_sb = inpool.tile([H, G, W], mybir.dt.float32)
            nc.sync.dma_start(out=l_sb[:], in_=l.ap().rearrange("g h w -> h g w"))
            nc.sync.dma_start(out=r_sb[:], in_=r.ap().rearrange("g h w -> h g w"))
            for d in range(MAX_DISP):
                t = outpool.tile([H, G, W], mybir.dt.float32)
                if d > 0:
                    nc.vector.memset(t[:, :, :d], 1.0)
                nc.vector.tensor_sub(t[:, :, d:], l_sb[:, :, d:], r_sb[:, :, : W - d])
                nc.sync.dma_start(
                    out=out.ap()[:, d, :, :].rearrange("g h w -> h g w"), in_=t[:]
                )

    nc.compile()
    _CACHE["nc"] = nc
    return nc


def make_in_maps(l_fmap, r_fmap):
    l_flat = np.ascontiguousarray(l_fmap, dtype=np.float32).reshape(N * C, H, W)
    r_flat = np.ascontiguousarray(r_fmap, dtype=np.float32).reshape(N * C, H, W)
    return [
        {
            "l": np.ascontiguousarray(l_flat[k * G : (k + 1) * G]),
            "r": np.ascontiguousarray(r_flat[k * G : (k + 1) * G]),
        }
        for k in range(NCORES)
    ]


def gather(results):
    out = np.concatenate([res["out"][None] for res in results], axis=0)
    return out.reshape(N, C, MAX_DISP, H, W)


def kernel(l_fmap, r_fmap):
    nc = build_bass()
    in_maps = make_in_maps(l_fmap, r_fmap)
    res = run_bass_kernel_spmd(nc, in_maps, core_ids=list(range(NCORES)))
    return gather(res.results)
